# revision 33
# baseline (speedup 1.0000x reference)
"""Trainium2 Bass kernel for batched multi-head attention (B=2, S=2048, E=1024, H=16).

Sharding: core r = 4*b + g handles batch b and head-group g (4 heads, 256 emb cols).
- QKV projections in fp16 (tensor-parallel over head groups).
- Attention per head: scores [keys, q] in PSUM, exp on ScalarE (fp16 out),
  PV accumulated over key blocks with an appended mask/64 column producing Z/64.
- Unnormalized contexts + Z rows are exchanged with one 8-rank AllToAll per head
  (cross-batch slots zeroed via dual evacuation with a host 0/1 column).
- Receiver normalizes: batched reciprocal of all 16 Z rows, gpsimd broadcast,
  then the 512-token out-proj slice in fp16.
"""

import sys

if '/opt/trn_rl_repo' not in sys.path:
    sys.path.insert(0, '/opt/trn_rl_repo')

import numpy as np

P = 128
B, S, E, H, DH = 2, 2048, 1024, 16, 64
NCORES = 8
G = 4                 # head groups == cores per batch
EG = E // G           # 256 emb cols per group
TS = S // G           # 512 tokens per core in out-proj
KB = S // P           # 16 key-token blocks
IB = E // P           # 8 contraction blocks of 128
QW = 512              # matmul moving free-dim chunk
SCALE = DH ** -0.5
ZDIV = 64.0           # ones-column divisor: Z' = Z/ZDIV keeps fp16 range nice
SC8 = True            # scores matmul in fp8e4m3 DoubleRow (2x PE throughput)

_cache = {}


def _build():
    import concourse.mybir as mybir
    import concourse.tile as tile
    from concourse import bacc
    from contextlib import ExitStack

    f32 = mybir.dt.float32
    f16 = mybir.dt.float16
    f32r_ = mybir.dt.float32r
    f8 = mybir.dt.float8e4
    AF = mybir.ActivationFunctionType
    MUL = mybir.AluOpType.mult
    ADD = mybir.AluOpType.add

    nc = bacc.Bacc("TRN2", target_bir_lowering=False, debug=False,
                   num_devices=NCORES)

    # ---- DRAM I/O ----
    xT = {n: nc.dram_tensor(f"x{n}T", [E, S], f16, kind="ExternalInput").ap()
          for n in "qkv"}
    wT = {n: nc.dram_tensor(f"w{n}T", [E, EG], f16, kind="ExternalInput").ap()
          for n in "qkv"}
    woT = nc.dram_tensor("woT", [E, E], f16, kind="ExternalInput").ap()
    bpk = {n: nc.dram_tensor(f"b{n}pk", [P, 2], f32, kind="ExternalInput").ap()
           for n in "qkv"}
    bov = nc.dram_tensor("bov", [1, E], f32, kind="ExternalInput").ap()
    mask_pb = nc.dram_tensor("mask_pb", [P, KB], f32, kind="ExternalInput").ap()
    selmat = nc.dram_tensor("selmat", [IB, IB * P], f32r_,
                            kind="ExternalInput").ap()
    maskdiv = nc.dram_tensor("maskdiv", [P, KB * G], f32, kind="ExternalInput").ap()
    zab = nc.dram_tensor("zab", [DH + 1, 2], f32, kind="ExternalInput").ap()
    out = nc.dram_tensor("out", [TS, E], f32, kind="ExternalOutput").ap()
    dbg = {}
    if _cache.get('debug'):
        qkd = f8 if SC8 else f16
        for nm, shp, dt_ in [("d_qp", [2 * P, S], qkd), ("d_kp", [2 * P, S], qkd),
                             ("d_vptok", [P, KB * EG], f32),
                             ("d_es", [P, S], f16), ("d_pv", [DH + 1, S], f32),
                             ("d_ga0", [P, TS], f32), ("d_st0", [2 * P, TS], f16)]:
            dbg[nm] = nc.dram_tensor(nm, shp, dt_, kind="ExternalOutput").ap()

    a2a_ins = [nc.dram_tensor(f"a2a_in{h}", [NCORES, DH + 1, TS], f16).ap()
               for h in range(G)]
    a2a_outs = [nc.dram_tensor(f"a2a_out{h}", [NCORES, DH + 1, TS], f16).ap()
                for h in range(G)]

    with tile.TileContext(nc) as tc, ExitStack() as top:
        const = top.enter_context(tc.tile_pool(name="const", bufs=1))
        b_sb = {}
        for n in "qkv":
            t = const.tile([P, 2], f32, tag=f"b{n}", name=f"b{n}")
            nc.sync.dma_start(t[:], bpk[n][:])
            b_sb[n] = t
        mask_t = const.tile([P, KB], f32)
        nc.sync.dma_start(mask_t[:], mask_pb[:])
        maskdiv_t = const.tile([P, KB * G], f32)
        nc.sync.dma_start(maskdiv_t[:], maskdiv[:])
        zab_t = const.tile([DH + 1, 2], f32)
        nc.sync.dma_start(zab_t[:], zab[:])
        bo_row = const.tile([1, E], f32, tag="bo_row", name="bo_row")
        nc.sync.dma_start(bo_row[:], bov[:])
        boB = const.tile([P, E], f32, tag="boB", name="boB")
        nc.gpsimd.partition_broadcast(boB[:], bo_row[:])
        sel_t = const.tile([IB, IB * P], f32r_, tag="selmat", name="selmat")
        nc.sync.dma_start(sel_t[:], selmat[:])

        # ---- projection weights: one consolidated DMA per tensor ----
        w_pool = top.enter_context(tc.tile_pool(name="wqkv", bufs=1))
        w_sb = {}
        for n in "vkq":
            t = w_pool.tile([P, IB * EG], f16, tag=f"w{n}", name=f"w{n}")
            nc.sync.dma_start(t.rearrange("p (i e) -> p i e", e=EG),
                              wT[n].rearrange("(i p) e -> p i e", p=P))
            w_sb[n] = t

        # persistent projection outputs
        proj_sb = top.enter_context(tc.tile_pool(name="proj_sb", bufs=1))
        qk_dt = f8 if SC8 else f16
        qp_sb = [proj_sb.tile([P, S], qk_dt, tag=f"qp{m}", name=f"qp{m}")
                 for m in range(2)]
        kp_sb = [proj_sb.tile([P, S], qk_dt, tag=f"kp{m}", name=f"kp{m}")
                 for m in range(2)]
        if SC8:
            # folded DoubleRow layouts per head: [32, ko=2, S] fp8,
            # head dim d = ki + 32*ko
            qpf = [proj_sb.tile([DH // 2, 2 * S], f8, tag=f"qpf{h}",
                                name=f"qpf{h}") for h in range(G)]
            kpf = [proj_sb.tile([DH // 2, 2 * S], f8, tag=f"kpf{h}",
                                name=f"kpf{h}") for h in range(G)]
        # vp tiles: per key-block, [P, 4 heads x (64 vals + 1 Z col)]
        vp_sb = [proj_sb.tile([P, G * (DH + 1)], f16, tag=f"vp{m}", name=f"vp{m}")
                 for m in range(KB)]

        # ---- projections: V fully first (incl. transpose), then K, Q ----
        with tc.tile_pool(name="vpT_p", bufs=1) as vpT_p, \
             tc.tile_pool(name="vtok_p", bufs=1) as vtok_p, \
             tc.tile_pool(name="xst", bufs=1) as xst, \
             tc.tile_pool(name="ppsum", bufs=1, space="PSUM") as ppsum:
            vpT = [vpT_p.tile([P, S], f32, tag=f"vpT{m}", name=f"vpT{m}")
                   for m in range(2)]

            def proj(n):
                xall = xst.tile([P, IB * S], f16, tag=f"x{n}", name=f"x{n}")
                for hf in range(2):
                    nc.sync.dma_start(
                        xall[:, hf * 4 * S:(hf + 1) * 4 * S]
                        .rearrange("p (i s) -> p i s", s=S),
                        xT[n].rearrange("(i p) s -> p i s", p=P)[:, hf * 4:(hf + 1) * 4, :])
                pss = [ppsum.tile([P, S], f32, tag=f"pp{m}", name=f"pp{m}")
                       for m in range(2)]
                for i in range(IB):
                    for m in range(2):
                        for c in range(S // QW):
                            nc.tensor.matmul(
                                pss[m][:, c * QW:(c + 1) * QW],
                                w_sb[n][:, i * EG + m * P:i * EG + (m + 1) * P],
                                xall[:, i * S + c * QW:i * S + (c + 1) * QW],
                                start=(i == 0), stop=(i == IB - 1))
                for m in range(2):
                    dst = {"v": vpT, "k": kp_sb, "q": qp_sb}[n][m]
                    nc.vector.tensor_scalar_add(dst[:], pss[m][:],
                                                b_sb[n][:, m:m + 1])

            proj("v")
            # ---- transpose vpT [e,t] -> token-major vp_sb tiles (on DVE,
            # overlapped with the K/Q projection matmuls below) ----
            vp_tok = vtok_p.tile([P, KB * EG], f32)
            SQ = 32
            for m2 in range(2):
                for a in range(P // SQ):
                    for b_ in range(P // SQ):
                        dst = vp_tok[a * SQ:(a + 1) * SQ, :] \
                            .rearrange("p (kt e) -> p kt e", e=EG)[
                                :, :, m2 * P + b_ * SQ:m2 * P + (b_ + 1) * SQ]
                        srcb = vpT[m2][b_ * SQ:(b_ + 1) * SQ, :] \
                            .rearrange("p (kt t) -> p kt t", t=P)[
                                :, :, a * SQ:(a + 1) * SQ]
                        nc.vector.transpose(dst, srcb)
            if dbg:
                nc.sync.dma_start(dbg["d_vptok"][:], vp_tok[:])
            for j in range(KB):
                src3 = vp_tok[:, j * EG:(j + 1) * EG] \
                    .rearrange("p (h d) -> p h d", h=G)
                dst3 = vp_sb[j].rearrange("p (h e) -> p h e", e=DH + 1)
                nc.vector.tensor_scalar_mul(dst3[:, :, 0:DH], src3,
                                            mask_t[:, j:j + 1])
                nc.vector.tensor_copy(
                    dst3[:, :, DH:DH + 1],
                    maskdiv_t[:, j * G:(j + 1) * G]
                    .rearrange("p (h e) -> p h e", e=1))
            proj("k")
            proj("q")
            if SC8:
                # fold [64, S] head blocks into [32, ko=2, S] via SBUF->SBUF
                # partition-offset copies (d = ki + 32*ko)
                for h in range(G):
                    m, u = h // 2, h % 2
                    for src_sb, dst_f in ((kp_sb, kpf), (qp_sb, qpf)):
                        for ko in range(2):
                            nc.sync.dma_start(
                                dst_f[h][:, ko * S:(ko + 1) * S],
                                src_sb[m][u * DH + ko * (DH // 2):
                                          u * DH + (ko + 1) * (DH // 2), :])

        if dbg:
            for m in range(2):
                nc.sync.dma_start(dbg["d_qp"][m * P:(m + 1) * P, :], qp_sb[m][:])
                nc.sync.dma_start(dbg["d_kp"][m * P:(m + 1) * P, :], kp_sb[m][:])

        # ---- out-proj weights: loaded during attention ----
        wo_pool = top.enter_context(tc.tile_pool(name="wo", bufs=1))
        wo_sb = wo_pool.tile([P, IB * E], f16)
        nc.sync.dma_start(wo_sb.rearrange("p (i e) -> p i e", e=E),
                          woT.rearrange("(i p) e -> p i e", p=P))

        # ---- attention ----
        with tc.tile_pool(name="sppsum", bufs=2, space="PSUM") as sppsum, \
             tc.tile_pool(name="pvpsum", bufs=1, space="PSUM") as pvpsum, \
             tc.tile_pool(name="expp", bufs=3) as expp, \
             tc.tile_pool(name="sendp", bufs=2) as sendp:
            for h in range(G):
                m = h // 2
                po = (h % 2) * DH
                pv = pvpsum.tile([DH + 1, S], f32)
                for j in range(KB):
                    es = expp.tile([P, S], f16)
                    for half in range(2):
                        sp = sppsum.tile([P, S // 2], f32)
                        for c in range(2):
                            q0 = half * (S // 2) + c * QW
                            if SC8:
                                nc.tensor.matmul(
                                    sp[:, c * QW:(c + 1) * QW],
                                    kpf[h].rearrange("p (ko t) -> p ko t", ko=2)
                                    [:, :, j * P:(j + 1) * P],
                                    qpf[h].rearrange("p (ko t) -> p ko t", ko=2)
                                    [:, :, q0:q0 + QW],
                                    start=True, stop=True,
                                    perf_mode=mybir.MatmulPerfMode.DoubleRow)
                            else:
                                nc.tensor.matmul(
                                    sp[:, c * QW:(c + 1) * QW],
                                    kp_sb[m][po:po + DH, j * P:(j + 1) * P],
                                    qp_sb[m][po:po + DH, q0:q0 + QW],
                                    start=True, stop=True)
                        nc.scalar.activation(
                            es[:, half * (S // 2):(half + 1) * (S // 2)],
                            sp[:], AF.Exp, scale=SCALE)
                    for c in range(S // QW):
                        nc.tensor.matmul(
                            pv[:, c * QW:(c + 1) * QW],
                            vp_sb[j][:, h * (DH + 1):(h + 1) * (DH + 1)],
                            es[:, c * QW:(c + 1) * QW],
                            start=(j == 0), stop=(j == KB - 1))
                if dbg and h == 0:
                    nc.sync.dma_start(dbg["d_es"][:], es[:])
                    pvdump = sendp.tile([DH + 1, S], f32, tag="pvd", name="pvd")
                    nc.vector.tensor_copy(pvdump[:], pv[:])
                    nc.sync.dma_start(dbg["d_pv"][:], pvdump[:])
                # dual evacuation: pvsA feeds slots 0-3 (batch-0 receivers),
                # pvsB slots 4-7; the host zab column zeroes the wrong batch.
                pvsA = sendp.tile([DH + 1, S], f16, tag="pvsA", name="pvsA")
                pvsB = sendp.tile([DH + 1, S], f16, tag="pvsB", name="pvsB")
                nc.vector.tensor_scalar_mul(pvsA[:], pv[:], zab_t[:, 0:1])
                nc.vector.tensor_scalar_mul(pvsB[:], pv[:], zab_t[:, 1:2])
                for sidx in range(G):
                    nc.sync.dma_start(a2a_ins[h][sidx],
                                      pvsA[:, sidx * TS:(sidx + 1) * TS])
                    nc.sync.dma_start(a2a_ins[h][G + sidx],
                                      pvsB[:, sidx * TS:(sidx + 1) * TS])
                nc.gpsimd.collective_compute(
                    "AllToAll", mybir.AluOpType.bypass,
                    replica_groups=[list(range(NCORES))],
                    ins=[a2a_ins[h][:]], outs=[a2a_outs[h][:]])

        # ---- gather + normalize + out-proj ----
        with tc.tile_pool(name="gap", bufs=1) as gap, \
             tc.tile_pool(name="recp", bufs=2, space="PSUM") as recp, \
             tc.tile_pool(name="opsum", bufs=2, space="PSUM") as opsum, \
             tc.tile_pool(name="outsb", bufs=2) as outsb:
            # Z' reciprocals per head-local pair: rzt[pair][4*w+g'] is
            # 1/Z' of head 4g'+(2*pair+w), ready after a2a 2*pair+1.
            # Emit pair 0 + even-ib gather first so that work runs while
            # the last a2a is still in flight; odd ibs depend on a2a 3.
            rzt = [None, None]
            ga = {}

            def zprep(pair):
                zi0 = gap.tile([NCORES, TS], f16, tag=f"zi0{pair}",
                               name=f"zi0{pair}")
                zi1 = gap.tile([NCORES, TS], f16, tag=f"zi1{pair}",
                               name=f"zi1{pair}")
                for w in range(2):
                    hl = 2 * pair + w
                    nc.sync.dma_start(zi0[G * w:G * w + G, :],
                                      a2a_outs[hl][0:G, DH:DH + 1, :])
                    nc.sync.dma_start(zi1[G * w:G * w + G, :],
                                      a2a_outs[hl][G:NCORES, DH:DH + 1, :])
                zsum = gap.tile([NCORES, TS], f32, tag=f"zs{pair}",
                                name=f"zs{pair}")
                nc.vector.tensor_tensor(zsum[:], zi0[:], zi1[:], ADD)
                rzp = gap.tile([NCORES, TS], f32r_, tag=f"rz{pair}",
                               name=f"rz{pair}")
                with nc.allow_low_precision(reason="1/Z at f32r is plenty"):
                    nc.vector.reciprocal(rzp[:], zsum[:])
                rzt[pair] = rzp

            def gather(ib):
                gp, u0 = ib // 2, (2 * ib) % G
                stA = gap.tile([P, TS], f16, tag=f"stA{ib}", name=f"stA{ib}")
                stB = gap.tile([P, TS], f16, tag=f"stB{ib}", name=f"stB{ib}")
                for u in range(2):
                    hl = u0 + u
                    nc.sync.dma_start(stA[u * DH:(u + 1) * DH, :],
                                      a2a_outs[hl][gp, 0:DH, :])
                    nc.sync.dma_start(stB[u * DH:(u + 1) * DH, :],
                                      a2a_outs[hl][gp + G, 0:DH, :])
                # recB[p, t] = 1/Z'(head 2ib + p//64, token t) via selector mm
                recB = recp.tile([P, TS], f32)
                nc.tensor.matmul(recB[:], sel_t[:, ib * P:(ib + 1) * P],
                                 rzt[ib % 2][:], start=True, stop=True)
                gsum = gap.tile([P, TS], f32, tag=f"gs{ib}", name=f"gs{ib}")
                nc.vector.tensor_tensor(gsum[:], stA[:], stB[:], ADD)
                gt = gap.tile([P, TS], f16, tag=f"ga{ib}", name=f"ga{ib}")
                nc.vector.tensor_tensor(gt[:], gsum[:], recB[:], MUL)
                ga[ib] = gt
                if dbg and ib == 0:
                    gd = gap.tile([P, TS], f32, tag="gad", name="gad")
                    nc.vector.tensor_copy(gd[:], gt[:])
                    nc.sync.dma_start(dbg["d_ga0"][:], gd[:])
                    nc.sync.dma_start(dbg["d_st0"][0:P, :], stA[:])
                    nc.sync.dma_start(dbg["d_st0"][P:2 * P, :], stB[:])

            zprep(0)
            for ib in (0, 2, 4, 6):
                gather(ib)
            zprep(1)
            for ib in (1, 3, 5, 7):
                gather(ib)

            ib_order = (0, 2, 4, 6, 1, 3, 5, 7)
            for tm in range(TS // P):
                pot = opsum.tile([P, E], f32)
                for n_, ib in enumerate(ib_order):
                    for oc in range(E // QW):
                        nc.tensor.matmul(
                            pot[:, oc * QW:(oc + 1) * QW],
                            ga[ib][:, tm * P:(tm + 1) * P],
                            wo_sb[:, ib * E + oc * QW:ib * E + oc * QW + QW],
                            start=(n_ == 0), stop=(n_ == IB - 1))
                ot = outsb.tile([P, E], f32)
                nc.vector.scalar_tensor_tensor(ot[:], pot[:], 1.0 / ZDIV,
                                               boB[:], MUL, ADD)
                nc.sync.dma_start(out[tm * P:(tm + 1) * P, :], ot[:])

    nc.compile()
    return nc


def _get_nc():
    if 'nc' not in _cache:
        _cache['nc'] = _build()
    return _cache['nc']


def kernel(q, k, v, mask, Wq, bq, Wk, bk, Wv, bv, Wo, bo):
    from concourse.bass_utils import run_bass_kernel_spmd

    nc = _get_nc()
    f32, f16 = np.float32, np.float16

    q = np.asarray(q, f32)
    k = np.asarray(k, f32)
    v = np.asarray(v, f32)
    qT = [np.ascontiguousarray(q[b].T).astype(f16) for b in range(B)]
    kT = [np.ascontiguousarray(k[b].T).astype(f16) for b in range(B)]
    vT = [np.ascontiguousarray(v[b].T).astype(f16) for b in range(B)]
    WqT = np.asarray(Wq, f32).T.astype(f16)
    WkT = np.asarray(Wk, f32).T.astype(f16)
    WvT = np.asarray(Wv, f32).T.astype(f16)
    WoT = np.ascontiguousarray(np.asarray(Wo, f32).T).astype(f16)
    bq = np.asarray(bq, f32)
    bk = np.asarray(bk, f32)
    bv = np.asarray(bv, f32)
    bo = np.asarray(bo, f32)
    maskf = (np.asarray(mask) != 0).astype(f32)  # [B, S]
    selm = np.zeros((IB, IB * P), f32)
    for ib_ in range(IB):
        for p_ in range(P):
            selm[4 * (p_ // DH) + ib_ // 2, ib_ * P + p_] = 1.0

    in_maps = []
    for r in range(NCORES):
        b, g = r // G, r % G
        cols = slice(g * EG, (g + 1) * EG)
        m_pb = np.ascontiguousarray(maskf[b].reshape(KB, P).T)        # [128,16]
        m_div = np.ascontiguousarray(np.repeat(m_pb, G, axis=1)) / ZDIV
        za = np.zeros((DH + 1, 2), f32)
        za[:, b] = 1.0
        in_maps.append({
            "xqT": qT[b], "xkT": kT[b], "xvT": vT[b],
            "wqT": np.ascontiguousarray(WqT[:, cols]),
            "wkT": np.ascontiguousarray(WkT[:, cols]),
            "wvT": np.ascontiguousarray(WvT[:, cols]),
            "woT": WoT,
            "bqpk": np.ascontiguousarray(bq[cols].reshape(2, P).T),
            "bkpk": np.ascontiguousarray(bk[cols].reshape(2, P).T),
            "bvpk": np.ascontiguousarray(bv[cols].reshape(2, P).T),
            "bov": bo[None, :],
            "mask_pb": m_pb, "maskdiv": m_div, "zab": za, "selmat": selm,
        })

    res = run_bass_kernel_spmd(nc, in_maps, core_ids=list(range(NCORES)),
                               **_cache.get('run_kwargs', {}))
    _cache['last_results'] = res

    full = np.empty((B, S, E), f32)
    for r in range(NCORES):
        b, g = r // G, r % G
        full[b, g * TS:(g + 1) * TS, :] = res.results[r]["out"]
    return full
